# revision 17
# baseline (speedup 1.0000x reference)
"""DeepSet kernel for Trainium2 (8 NeuronCores, data-parallel).

Model (reference):
    mask  = sign(|sum_e words|)                  # padding rows are all-zero
    h1    = tanh(words @ W1 + b1)                # [B,S,H]
    h2    = tanh(h1 @ W2 + b2)                   # [B,S,H]
    enc   = h2 @ W3 + b3                         # [B,S,C]
    codes = sum_s enc * mask                     # [B,C]
    out   = (tanh(tanh(codes@W4+b4)@W5+b5)) @ W6 + b6   # [B,T]

Algebraic collapse: codes = (sum_s mask*h2) @ W3 + N_b * b3, so only the two
big MLP layers run on device; the tiny decode runs on host.

Layout strategy (v2): BOTH layers produce transposed activations [h, r]
(h on partitions, rows moving).  This makes the per-feature biases b1/b2
per-PARTITION quantities, so the ScalarE activation instruction applies
bias+tanh in one pass — no VectorE bias add at all.  The segment sum
(rows are packed contiguously per set) becomes a free-axis segmented
reduction over a2[h, r] handled by the idle VectorE — the 38 PE segment
matmuls of v1 are gone.  Everything streams in bf16 (rel err ~3e-3,
gate is 2e-2): halves input DMA and makes LDWEIGHTS (150ns) fully
hidden behind each 512-moving matmul (fp32 LDWEIGHTS was 330ns > 213ns
matmul and leaked into the critical path).

Per 512-row tile (R rows/core, packed valid rows, zero pad):
    L1: for m (h-chunk): ps1[m][h,r] = sum_k W1[k,m]^T a0[k]   (4 mm)
        a1[m] = tanh(ps1[m] + b1[m])        ScalarE, per-partition bias
    L2: for m: for n: ps2[n][h,r] += W2[m,n]^T a1[m]           (16 mm)
        a2[n] = tanh(ps2[n] + b2[n])        ScalarE
    seg: for each set-piece [lo,hi) in tile: VectorE reduce_sum over
        a2[n][:, lo:hi] -> outsb[:, n, t, j]; host scatters to sets.
PSUM: 4 banks L1 + 4 banks L2 = 8 (all).  m-outer ordering lets L2(t)
matmuls start while L1(t) activations finish (no PE bubble).
"""

import sys

if "/opt/trn_rl_repo" not in sys.path:
    sys.path.insert(0, "/opt/trn_rl_repo")

import ml_dtypes
import numpy as np

import concourse.bass as bass
import concourse.mybir as mybir
import concourse.tile as tile
from concourse import bacc
from concourse.bass_utils import run_bass_kernel_spmd

B, S, E = 64, 1024, 512
H = 512
NCORES = 8
P = 128
RT = 512  # rows per row-tile (matmul moving dim)
KC = E // P  # 4 contraction chunks

N_WARMUP = 8  # dep-free matmuls to open the HAM clock gate during DMA wait

f32 = mybir.dt.float32
bf16 = mybir.dt.bfloat16

_cache: dict = {}


def _tiles_of(R: int):
    assert R % 256 == 0 and R >= RT
    return [RT] * (R // RT) + ([256] if R % RT else [])


def _build(R: int, pieces_key):
    """pieces_key: tuple over tiles of tuples (lo, hi) per piece."""
    key = (R, pieces_key)
    if key in _cache:
        return _cache[key]

    tiles = _tiles_of(R)
    nt = len(tiles)
    offs = [sum(tiles[:i]) for i in range(nt)]
    pmax = max(1, max((len(p) for p in pieces_key), default=1))

    nc = bacc.Bacc("TRN2", target_bir_lowering=False, debug=False, num_devices=NCORES)

    wT_d = nc.dram_tensor("wT", [P, KC, R], bf16, kind="ExternalInput").ap()
    w1_d = nc.dram_tensor("w1", [P, KC, H], bf16, kind="ExternalInput").ap()
    w2_d = nc.dram_tensor("w2", [P, KC, H], bf16, kind="ExternalInput").ap()
    b1_d = nc.dram_tensor("b1", [P, KC], f32, kind="ExternalInput").ap()
    b2_d = nc.dram_tensor("b2", [P, KC], f32, kind="ExternalInput").ap()
    out_d = nc.dram_tensor("hsum", [nt, P, KC, pmax], f32, kind="ExternalOutput").ap()

    with tile.TileContext(nc) as tc:
        with (
            tc.tile_pool(name="const", bufs=1) as cpool,
            tc.tile_pool(name="a0", bufs=3) as a0pool,
            tc.tile_pool(name="a1", bufs=2) as a1pool,
            tc.tile_pool(name="a2", bufs=3) as a2pool,
            tc.tile_pool(name="ps1", bufs=4, space="PSUM") as ps1pool,
            tc.tile_pool(name="ps2", bufs=4, space="PSUM") as ps2pool,
        ):
            # --- DMA issue order = critical path first ---------------------
            # Sync engine issues DMA descriptors serially (~0.7us each), so
            # the first L1 matmul's deps (w1[k0], a0[t0][k0]) go first, in
            # k-pair order matching the t0 k-outer matmul schedule.
            w1k = []
            a0t0 = []
            for k in range(KC):
                w1c = cpool.tile([P, H], bf16, name=f"w1k{k}")
                nc.sync.dma_start(w1c[:], w1_d[:, k, :])
                w1k.append(w1c)
                a0c = cpool.tile([P, RT], bf16, name=f"a0t0k{k}")
                nc.sync.dma_start(a0c[:], wT_d[:, k, offs[0]:offs[0] + tiles[0]])
                a0t0.append(a0c)
            b1sb = cpool.tile([P, KC], f32)
            nc.sync.dma_start(b1sb[:], b1_d)
            b2sb = cpool.tile([P, KC], f32)
            nc.sync.dma_start(b2sb[:], b2_d)
            w2k = []
            for k in range(KC):
                w2c = cpool.tile([P, H], bf16, name=f"w2k{k}")
                nc.sync.dma_start(w2c[:], w2_d[:, k, :])
                w2k.append(w2c)
            # remaining row-tiles: one combined 4-chunk DMA each; the pool
            # (bufs=3) back-pressures the Sync queue harmlessly — everything
            # urgent is already issued above.
            a0t = {}
            for t in range(1, nt):
                a0c = a0pool.tile([P, KC, RT], bf16, tag="a0", name=f"a0t{t}")
                nc.sync.dma_start(
                    a0c[:, :, :tiles[t]], wT_d[:, :, offs[t]:offs[t] + tiles[t]]
                )
                a0t[t] = a0c

            # --- PE warmup: open the HAM clock gate during DMA wait --------
            # (measured: the gate opens ~4.3us after ~3.4us of continuous PE
            # activity, right as the first input DMAs land)
            warm_sb = cpool.tile([P, RT], bf16)
            nc.gpsimd.memset(warm_sb[:], 0.25)
            for w in range(N_WARMUP):
                wps = ps1pool.tile([P, RT], f32, tag="ps1", name="wps")
                nc.tensor.matmul(
                    wps[:], warm_sb[:, :P], warm_sb[:], start=True, stop=True
                )

            # --- main pipeline --------------------------------------------
            for t in range(nt):
                nr = tiles[t]
                pieces = pieces_key[t]
                # L1 -> a1[m] = tanh(W1[:,m]^T a0 + b1[m]), layout [h, r]
                a1 = [
                    a1pool.tile([P, RT], bf16, tag=f"a1m{m}", name=f"a1m{m}")
                    for m in range(KC)
                ]
                if t == 0:
                    # k-outer: each arriving (w1[k], a0[k]) DMA pair feeds 4
                    # matmuls immediately
                    pss = [
                        ps1pool.tile([P, RT], f32, tag="ps1", name=f"ps1_{m}")
                        for m in range(KC)
                    ]
                    for k in range(KC):
                        for m in range(KC):
                            nc.tensor.matmul(
                                pss[m][:, :nr],
                                w1k[k][:, m * P:(m + 1) * P],
                                a0t0[k][:, :nr],
                                start=(k == 0),
                                stop=(k == KC - 1),
                            )
                    for m in range(KC):
                        nc.scalar.activation(
                            a1[m][:, :nr], pss[m][:, :nr],
                            mybir.ActivationFunctionType.Tanh,
                            bias=b1sb[:, m:m + 1],
                        )
                else:
                    a0c = a0t[t]
                    for m in range(KC):
                        ps = ps1pool.tile([P, RT], f32, tag="ps1")
                        for k in range(KC):
                            nc.tensor.matmul(
                                ps[:, :nr],
                                w1k[k][:, m * P:(m + 1) * P],
                                a0c[:, k, :nr],
                                start=(k == 0),
                                stop=(k == KC - 1),
                            )
                        nc.scalar.activation(
                            a1[m][:, :nr], ps[:, :nr],
                            mybir.ActivationFunctionType.Tanh,
                            bias=b1sb[:, m:m + 1],
                        )
                # L2: m-outer so the first matmuls only need a1[0] (no PE
                # bubble waiting on L1 activations).  The last tile runs
                # n-outer instead: each psum bank finishes early so its
                # activation overlaps remaining matmuls (shorter kernel tail;
                # a1 is fully ready there anyway).
                ps2 = [
                    ps2pool.tile([P, RT], f32, tag="ps2", name=f"ps2_{n}")
                    for n in range(KC)
                ]
                a2 = a2pool.tile([P, KC, RT], bf16, tag="a2", name="a2")
                if t == nt - 1:
                    for n in range(KC):
                        for m in range(KC):
                            nc.tensor.matmul(
                                ps2[n][:, :nr],
                                w2k[m][:, n * P:(n + 1) * P],
                                a1[m][:, :nr],
                                start=(m == 0),
                                stop=(m == KC - 1),
                            )
                        nc.scalar.activation(
                            a2[:, n, :nr], ps2[n][:, :nr],
                            mybir.ActivationFunctionType.Tanh,
                            bias=b2sb[:, n:n + 1],
                        )
                else:
                    for m in range(KC):
                        for n in range(KC):
                            nc.tensor.matmul(
                                ps2[n][:, :nr],
                                w2k[m][:, n * P:(n + 1) * P],
                                a1[m][:, :nr],
                                start=(m == 0),
                                stop=(m == KC - 1),
                            )
                    for n in range(KC):
                        nc.scalar.activation(
                            a2[:, n, :nr], ps2[n][:, :nr],
                            mybir.ActivationFunctionType.Tanh,
                            bias=b2sb[:, n:n + 1],
                        )
                # segment partial sums, all 4 h-chunks per instr (VectorE)
                outsb = a1pool.tile([P, KC, pmax], f32, tag="out", name="outsb")
                for j, (lo, hi) in enumerate(pieces):
                    nc.vector.reduce_sum(
                        outsb[:, :, j],
                        a2[:, :, lo:hi],
                        axis=mybir.AxisListType.X,
                    )
                nc.sync.dma_start(out_d[t], outsb[:])

    nc.compile()
    _cache[key] = nc
    return nc


def _pack(words: np.ndarray):
    """Pack valid rows contiguously, split across cores.

    Returns per-core (wT bf16 [P,KC,R], pieces [(lo,hi,gid) per tile]) plus
    R and the validity mask.
    """
    words = np.asarray(words, dtype=np.float32)
    mask = np.sign(np.abs(words.sum(axis=-1)))  # [B, S], matches reference
    valid = mask > 0

    rows = []
    segs = []
    for b in range(B):
        vb = words[b][valid[b]]
        rows.append(vb)
        segs.append(np.full(len(vb), b, dtype=np.int64))
    rows = np.concatenate(rows, axis=0)
    segs = np.concatenate(segs, axis=0)
    total = len(rows)

    # balanced split: per-core valid counts differ by <=1, so the tail
    # boundaries of all cores collapse to at most two span offsets
    base, extra = divmod(total, NCORES)
    counts = [base + (1 if c < extra else 0) for c in range(NCORES)]
    starts = np.concatenate([[0], np.cumsum(counts)])
    R = max(RT, -(-max(counts) // 256) * 256)  # pad to tile granularity
    tiles = _tiles_of(R)
    offs = [sum(tiles[:i]) for i in range(len(tiles))]

    per_core = []
    for c in range(NCORES):
        lo, hi = int(starts[c]), int(starts[c + 1])
        chunk = rows[lo:hi]
        seg_chunk = segs[lo:hi]
        n = hi - lo
        if n < R:
            chunk = np.concatenate(
                [chunk, np.zeros((R - n, E), dtype=np.float32)], axis=0
            )
        wT = np.ascontiguousarray(
            chunk.T.reshape(KC, P, R).transpose(1, 0, 2)
        ).astype(ml_dtypes.bfloat16)  # [P, KC, R]
        # set pieces per tile: maximal runs of equal set id among valid rows
        pieces = []
        for t, nr in enumerate(tiles):
            t_lo, t_hi = offs[t], min(offs[t] + nr, n)
            pt = []
            i = t_lo
            while i < t_hi:
                g = seg_chunk[i]
                j = i
                while j < t_hi and seg_chunk[j] == g:
                    j += 1
                pt.append((i - t_lo, j - t_lo, int(g)))
                i = j
            pieces.append(pt)
        per_core.append((wT, pieces))
    return per_core, R, mask


def _in_maps(per_core, inputs):
    W1 = np.asarray(inputs["W1"], dtype=np.float32)
    W2 = np.asarray(inputs["W2"], dtype=np.float32)
    b1 = np.asarray(inputs["b1"], dtype=np.float32)
    b2 = np.asarray(inputs["b2"], dtype=np.float32)
    w1r = np.ascontiguousarray(
        W1.reshape(KC, P, H).transpose(1, 0, 2)
    ).astype(ml_dtypes.bfloat16)  # [P, KC, H], [p,k,h] = W1[k*P+p, h]
    w2r = np.ascontiguousarray(
        W2.reshape(KC, P, H).transpose(1, 0, 2)
    ).astype(ml_dtypes.bfloat16)
    b1r = np.ascontiguousarray(b1.reshape(KC, P).T)  # [p, m] = b1[m*P+p]
    b2r = np.ascontiguousarray(b2.reshape(KC, P).T)
    return [
        {"wT": wT, "w1": w1r, "w2": w2r, "b1": b1r, "b2": b2r}
        for (wT, _pieces) in per_core
    ]


def _prepare(words):
    per_core, R, mask = _pack(words)
    # All cores share one SPMD program, but set boundaries differ per core.
    # Per tile position, collect the union of all cores' piece boundary
    # offsets and emit one reduce per consecutive-offset span: no span ever
    # straddles any core's set boundary, so each core's piece is an exact
    # union of spans (host sums the slots it needs and ignores the rest).
    pieces_key = []
    nt = len(_tiles_of(R))
    for t in range(nt):
        offsets = set()
        for (_wT, pieces) in per_core:
            for (lo, hi, _g) in pieces[t]:
                offsets.add(lo)
                offsets.add(hi)
        offs_sorted = sorted(offsets)
        spans = tuple(
            (a, b) for a, b in zip(offs_sorted, offs_sorted[1:])
        )
        pieces_key.append(spans)
    pieces_key = tuple(pieces_key)
    nc = _build(R, pieces_key)
    return per_core, pieces_key, R, mask, nc


def kernel(words, W1, b1, W2, b2, W3, b3, W4, b4, W5, b5, W6, b6):
    per_core, pieces_key, R, mask, nc = _prepare(words)
    in_maps = _in_maps(per_core, {"W1": W1, "W2": W2, "b1": b1, "b2": b2})

    res = run_bass_kernel_spmd(nc, in_maps, core_ids=list(range(NCORES)))

    hsum = np.zeros((B, H), dtype=np.float32)
    for c in range(NCORES):
        out_c = res.results[c]["hsum"]  # [nt, P, KC, pmax]
        hvec = out_c.transpose(2, 1, 0, 3).reshape(H, out_c.shape[0], -1)
        pieces = per_core[c][1]
        for t, pt in enumerate(pieces):
            for (lo, hi, g) in pt:
                for j, (slo, shi) in enumerate(pieces_key[t]):
                    if lo <= slo and shi <= hi:
                        hsum[g] += hvec[:, t, j]

    # host decode (tiny)
    lengths = mask.sum(axis=1).astype(np.float32)[:, None]
    codes = hsum @ np.asarray(W3, np.float32) + lengths * np.asarray(b3, np.float32)
    h = np.tanh(codes @ np.asarray(W4, np.float32) + np.asarray(b4, np.float32))
    h = np.tanh(h @ np.asarray(W5, np.float32) + np.asarray(b5, np.float32))
    out = h @ np.asarray(W6, np.float32) + np.asarray(b6, np.float32)
    return out.astype(np.float32)


# revision 21
# speedup vs baseline: 1.0215x; 1.0215x over previous
"""DeepSet kernel for Trainium2 (8 NeuronCores, data-parallel).

Model (reference):
    mask  = sign(|sum_e words|)                  # padding rows are all-zero
    h1    = tanh(words @ W1 + b1)                # [B,S,H]
    h2    = tanh(h1 @ W2 + b2)                   # [B,S,H]
    enc   = h2 @ W3 + b3                         # [B,S,C]
    codes = sum_s enc * mask                     # [B,C]
    out   = (tanh(tanh(codes@W4+b4)@W5+b5)) @ W6 + b6   # [B,T]

Algebraic collapse: codes = (sum_s mask*h2) @ W3 + N_b * b3, so only the two
big MLP layers run on device; the tiny decode runs on host.

Layout strategy (v2): BOTH layers produce transposed activations [h, r]
(h on partitions, rows moving).  This makes the per-feature biases b1/b2
per-PARTITION quantities, so the ScalarE activation instruction applies
bias+tanh in one pass — no VectorE bias add at all.  The segment sum
(rows are packed contiguously per set) becomes a free-axis segmented
reduction over a2[h, r] handled by the idle VectorE — the 38 PE segment
matmuls of v1 are gone.  Everything streams in bf16 (rel err ~3e-3,
gate is 2e-2): halves input DMA and makes LDWEIGHTS (150ns) fully
hidden behind each 512-moving matmul (fp32 LDWEIGHTS was 330ns > 213ns
matmul and leaked into the critical path).

Per 512-row tile (R rows/core, packed valid rows, zero pad):
    L1: for m (h-chunk): ps1[m][h,r] = sum_k W1[k,m]^T a0[k]   (4 mm)
        a1[m] = tanh(ps1[m] + b1[m])        ScalarE, per-partition bias
    L2: for m: for n: ps2[n][h,r] += W2[m,n]^T a1[m]           (16 mm)
        a2[n] = tanh(ps2[n] + b2[n])        ScalarE
    seg: for each set-piece [lo,hi) in tile: VectorE reduce_sum over
        a2[n][:, lo:hi] -> outsb[:, n, t, j]; host scatters to sets.
PSUM: 4 banks L1 + 4 banks L2 = 8 (all).  m-outer ordering lets L2(t)
matmuls start while L1(t) activations finish (no PE bubble).
"""

import sys

if "/opt/trn_rl_repo" not in sys.path:
    sys.path.insert(0, "/opt/trn_rl_repo")

import ml_dtypes
import numpy as np

import concourse.bass as bass
import concourse.mybir as mybir
import concourse.tile as tile
from concourse import bacc
from concourse.bass_utils import run_bass_kernel_spmd

B, S, E = 64, 1024, 512
H = 512
NCORES = 8
P = 128
RT = 512  # rows per row-tile (matmul moving dim)
KC = E // P  # 4 contraction chunks

N_WARMUP = 8  # dep-free matmuls to open the HAM clock gate during DMA wait

f32 = mybir.dt.float32
bf16 = mybir.dt.bfloat16

_cache: dict = {}


def _tiles_of(R: int):
    assert R % 256 == 0 and R >= RT
    return [RT] * (R // RT) + ([256] if R % RT else [])


def _build(R: int, pieces_key):
    """pieces_key: tuple over tiles of tuples (lo, hi) per piece."""
    key = (R, pieces_key)
    if key in _cache:
        return _cache[key]

    tiles = _tiles_of(R)
    nt = len(tiles)
    offs = [sum(tiles[:i]) for i in range(nt)]
    pmax = max(1, max((len(p) for p in pieces_key), default=1))

    nc = bacc.Bacc("TRN2", target_bir_lowering=False, debug=False, num_devices=NCORES)

    wT_d = nc.dram_tensor("wT", [P, KC, R], bf16, kind="ExternalInput").ap()
    w1_d = nc.dram_tensor("w1", [P, KC, H], bf16, kind="ExternalInput").ap()
    w2_d = nc.dram_tensor("w2", [P, KC, H], bf16, kind="ExternalInput").ap()
    bias_d = nc.dram_tensor("bias", [P, 2, KC], f32, kind="ExternalInput").ap()
    out_d = nc.dram_tensor("hsum", [nt, P, KC, pmax], f32, kind="ExternalOutput").ap()

    with tile.TileContext(nc) as tc:
        with (
            tc.tile_pool(name="const", bufs=1) as cpool,
            tc.tile_pool(name="a0", bufs=3) as a0pool,
            tc.tile_pool(name="a1", bufs=2) as a1pool,
            tc.tile_pool(name="a2", bufs=3) as a2pool,
            tc.tile_pool(name="ps1", bufs=4, space="PSUM") as ps1pool,
            tc.tile_pool(name="ps2", bufs=4, space="PSUM") as ps2pool,
        ):
            # --- DMA issue order = critical path first ---------------------
            # Sync engine issues DMA descriptors serially (~0.65us each).
            # Biases (one tiny 4KB transfer) go first: the very first tanh
            # needs them.  Then the first L1 matmul's deps (w1[k0],
            # a0[t0][k0]) in k-pair order matching the t0 k-outer schedule.
            bsb = cpool.tile([P, 2, KC], f32)
            nc.sync.dma_start(bsb[:], bias_d)
            b1sb = bsb[:, 0]
            b2sb = bsb[:, 1]
            w1k = []
            a0t0 = []
            for k in range(KC):
                w1c = cpool.tile([P, H], bf16, name=f"w1k{k}")
                nc.sync.dma_start(w1c[:], w1_d[:, k, :])
                w1k.append(w1c)
                a0c = cpool.tile([P, RT], bf16, name=f"a0t0k{k}")
                nc.sync.dma_start(a0c[:], wT_d[:, k, offs[0]:offs[0] + tiles[0]])
                a0t0.append(a0c)
            w2k = []
            for k in range(KC):
                w2c = cpool.tile([P, H], bf16, name=f"w2k{k}")
                nc.sync.dma_start(w2c[:], w2_d[:, k, :])
                w2k.append(w2c)
            # remaining row-tiles: one combined 4-chunk DMA each; the pool
            # (bufs=3) back-pressures the Sync queue harmlessly — everything
            # urgent is already issued above.
            a0t = {}
            for t in range(1, nt):
                a0c = a0pool.tile([P, KC, RT], bf16, tag="a0", name=f"a0t{t}")
                nc.sync.dma_start(
                    a0c[:, :, :tiles[t]], wT_d[:, :, offs[t]:offs[t] + tiles[t]]
                )
                a0t[t] = a0c

            # --- PE warmup: open the HAM clock gate during DMA wait --------
            # (measured: the gate opens ~4.3us after ~3.4us of continuous PE
            # activity, right as the first input DMAs land)
            warm_sb = cpool.tile([P, RT], bf16)
            nc.gpsimd.memset(warm_sb[:], 0.25)
            # dummy tanh: pulls the 1.3us ACT_TABLE_LOAD off the critical
            # path (otherwise it runs inline in the first real activation)
            warm_act = cpool.tile([P, 1], f32)
            nc.scalar.activation(
                warm_act[:], warm_sb[:, :1], mybir.ActivationFunctionType.Tanh
            )
            for w in range(N_WARMUP):
                wps = ps1pool.tile([P, RT], f32, tag="ps1", name="wps")
                nc.tensor.matmul(
                    wps[:], warm_sb[:, :P], warm_sb[:], start=True, stop=True
                )

            # --- main pipeline --------------------------------------------
            for t in range(nt):
                nr = tiles[t]
                pieces = pieces_key[t]
                # L1 -> a1[m] = tanh(W1[:,m]^T a0 + b1[m]), layout [h, r]
                a1 = [
                    a1pool.tile([P, RT], bf16, tag=f"a1m{m}", name=f"a1m{m}")
                    for m in range(KC)
                ]
                if t == 0:
                    # k-outer: each arriving (w1[k], a0[k]) DMA pair feeds 4
                    # matmuls immediately
                    pss = [
                        ps1pool.tile([P, RT], f32, tag="ps1", name=f"ps1_{m}")
                        for m in range(KC)
                    ]
                    for k in range(KC):
                        for m in range(KC):
                            nc.tensor.matmul(
                                pss[m][:, :nr],
                                w1k[k][:, m * P:(m + 1) * P],
                                a0t0[k][:, :nr],
                                start=(k == 0),
                                stop=(k == KC - 1),
                            )
                    for m in range(KC):
                        nc.scalar.activation(
                            a1[m][:, :nr], pss[m][:, :nr],
                            mybir.ActivationFunctionType.Tanh,
                            bias=b1sb[:, m:m + 1],
                        )
                else:
                    a0c = a0t[t]
                    for m in range(KC):
                        ps = ps1pool.tile([P, RT], f32, tag="ps1")
                        for k in range(KC):
                            nc.tensor.matmul(
                                ps[:, :nr],
                                w1k[k][:, m * P:(m + 1) * P],
                                a0c[:, k, :nr],
                                start=(k == 0),
                                stop=(k == KC - 1),
                            )
                        nc.scalar.activation(
                            a1[m][:, :nr], ps[:, :nr],
                            mybir.ActivationFunctionType.Tanh,
                            bias=b1sb[:, m:m + 1],
                        )
                # L2: m-outer so the first matmuls only need a1[0] (no PE
                # bubble waiting on L1 activations).  The last tile runs
                # n-outer instead: each psum bank finishes early so its
                # activation overlaps remaining matmuls (shorter kernel tail;
                # a1 is fully ready there anyway).
                ps2 = [
                    ps2pool.tile([P, RT], f32, tag="ps2", name=f"ps2_{n}")
                    for n in range(KC)
                ]
                a2 = a2pool.tile([P, KC, RT], bf16, tag="a2", name="a2")
                if t == nt - 1:
                    for n in range(KC):
                        for m in range(KC):
                            nc.tensor.matmul(
                                ps2[n][:, :nr],
                                w2k[m][:, n * P:(n + 1) * P],
                                a1[m][:, :nr],
                                start=(m == 0),
                                stop=(m == KC - 1),
                            )
                        nc.scalar.activation(
                            a2[:, n, :nr], ps2[n][:, :nr],
                            mybir.ActivationFunctionType.Tanh,
                            bias=b2sb[:, n:n + 1],
                        )
                else:
                    for m in range(KC):
                        for n in range(KC):
                            nc.tensor.matmul(
                                ps2[n][:, :nr],
                                w2k[m][:, n * P:(n + 1) * P],
                                a1[m][:, :nr],
                                start=(m == 0),
                                stop=(m == KC - 1),
                            )
                    for n in range(KC):
                        nc.scalar.activation(
                            a2[:, n, :nr], ps2[n][:, :nr],
                            mybir.ActivationFunctionType.Tanh,
                            bias=b2sb[:, n:n + 1],
                        )
                # segment partial sums, all 4 h-chunks per instr (VectorE)
                outsb = a1pool.tile([P, KC, pmax], f32, tag="out", name="outsb")
                for j, (lo, hi) in enumerate(pieces):
                    nc.vector.reduce_sum(
                        outsb[:, :, j],
                        a2[:, :, lo:hi],
                        axis=mybir.AxisListType.X,
                    )
                nc.sync.dma_start(out_d[t], outsb[:])

    nc.compile()
    _cache[key] = nc
    return nc


def _pack(words: np.ndarray):
    """Pack valid rows contiguously, split across cores.

    Returns per-core (wT bf16 [P,KC,R], pieces [(lo,hi,gid) per tile]) plus
    R and the validity mask.
    """
    words = np.asarray(words, dtype=np.float32)
    mask = np.sign(np.abs(words.sum(axis=-1)))  # [B, S], matches reference
    valid = mask > 0

    rows = []
    segs = []
    for b in range(B):
        vb = words[b][valid[b]]
        rows.append(vb)
        segs.append(np.full(len(vb), b, dtype=np.int64))
    rows = np.concatenate(rows, axis=0)
    segs = np.concatenate(segs, axis=0)
    total = len(rows)

    # balanced split: per-core valid counts differ by <=1, so the tail
    # boundaries of all cores collapse to at most two span offsets
    base, extra = divmod(total, NCORES)
    counts = [base + (1 if c < extra else 0) for c in range(NCORES)]
    starts = np.concatenate([[0], np.cumsum(counts)])
    R = max(RT, -(-max(counts) // 256) * 256)  # pad to tile granularity
    tiles = _tiles_of(R)
    offs = [sum(tiles[:i]) for i in range(len(tiles))]

    per_core = []
    for c in range(NCORES):
        lo, hi = int(starts[c]), int(starts[c + 1])
        chunk = rows[lo:hi]
        seg_chunk = segs[lo:hi]
        n = hi - lo
        if n < R:
            chunk = np.concatenate(
                [chunk, np.zeros((R - n, E), dtype=np.float32)], axis=0
            )
        wT = np.ascontiguousarray(
            chunk.T.reshape(KC, P, R).transpose(1, 0, 2)
        ).astype(ml_dtypes.bfloat16)  # [P, KC, R]
        # set pieces per tile: maximal runs of equal set id among valid rows
        pieces = []
        for t, nr in enumerate(tiles):
            t_lo, t_hi = offs[t], min(offs[t] + nr, n)
            pt = []
            i = t_lo
            while i < t_hi:
                g = seg_chunk[i]
                j = i
                while j < t_hi and seg_chunk[j] == g:
                    j += 1
                pt.append((i - t_lo, j - t_lo, int(g)))
                i = j
            pieces.append(pt)
        per_core.append((wT, pieces))
    return per_core, R, mask


def _in_maps(per_core, inputs):
    W1 = np.asarray(inputs["W1"], dtype=np.float32)
    W2 = np.asarray(inputs["W2"], dtype=np.float32)
    b1 = np.asarray(inputs["b1"], dtype=np.float32)
    b2 = np.asarray(inputs["b2"], dtype=np.float32)
    w1r = np.ascontiguousarray(
        W1.reshape(KC, P, H).transpose(1, 0, 2)
    ).astype(ml_dtypes.bfloat16)  # [P, KC, H], [p,k,h] = W1[k*P+p, h]
    w2r = np.ascontiguousarray(
        W2.reshape(KC, P, H).transpose(1, 0, 2)
    ).astype(ml_dtypes.bfloat16)
    b1r = b1.reshape(KC, P).T  # [p, m] = b1[m*P+p]
    b2r = b2.reshape(KC, P).T
    biasr = np.ascontiguousarray(np.stack([b1r, b2r], axis=1))  # [P, 2, KC]
    return [
        {"wT": wT, "w1": w1r, "w2": w2r, "bias": biasr}
        for (wT, _pieces) in per_core
    ]


def _prepare(words):
    per_core, R, mask = _pack(words)
    # All cores share one SPMD program, but set boundaries differ per core.
    # Per tile position, collect the union of all cores' piece boundary
    # offsets and emit one reduce per consecutive-offset span: no span ever
    # straddles any core's set boundary, so each core's piece is an exact
    # union of spans (host sums the slots it needs and ignores the rest).
    pieces_key = []
    nt = len(_tiles_of(R))
    for t in range(nt):
        offsets = set()
        for (_wT, pieces) in per_core:
            for (lo, hi, _g) in pieces[t]:
                offsets.add(lo)
                offsets.add(hi)
        offs_sorted = sorted(offsets)
        spans = tuple(
            (a, b) for a, b in zip(offs_sorted, offs_sorted[1:])
        )
        pieces_key.append(spans)
    pieces_key = tuple(pieces_key)
    nc = _build(R, pieces_key)
    return per_core, pieces_key, R, mask, nc


def kernel(words, W1, b1, W2, b2, W3, b3, W4, b4, W5, b5, W6, b6):
    per_core, pieces_key, R, mask, nc = _prepare(words)
    in_maps = _in_maps(per_core, {"W1": W1, "W2": W2, "b1": b1, "b2": b2})

    res = run_bass_kernel_spmd(nc, in_maps, core_ids=list(range(NCORES)))

    hsum = np.zeros((B, H), dtype=np.float32)
    for c in range(NCORES):
        out_c = res.results[c]["hsum"]  # [nt, P, KC, pmax]
        hvec = out_c.transpose(2, 1, 0, 3).reshape(H, out_c.shape[0], -1)
        pieces = per_core[c][1]
        for t, pt in enumerate(pieces):
            for (lo, hi, g) in pt:
                for j, (slo, shi) in enumerate(pieces_key[t]):
                    if lo <= slo and shi <= hi:
                        hsum[g] += hvec[:, t, j]

    # host decode (tiny)
    lengths = mask.sum(axis=1).astype(np.float32)[:, None]
    codes = hsum @ np.asarray(W3, np.float32) + lengths * np.asarray(b3, np.float32)
    h = np.tanh(codes @ np.asarray(W4, np.float32) + np.asarray(b4, np.float32))
    h = np.tanh(h @ np.asarray(W5, np.float32) + np.asarray(b5, np.float32))
    out = h @ np.asarray(W6, np.float32) + np.asarray(b6, np.float32)
    return out.astype(np.float32)


# revision 25
# speedup vs baseline: 1.0219x; 1.0004x over previous
"""DeepSet kernel for Trainium2 (8 NeuronCores, data-parallel).

Model (reference):
    mask  = sign(|sum_e words|)                  # padding rows are all-zero
    h1    = tanh(words @ W1 + b1)                # [B,S,H]
    h2    = tanh(h1 @ W2 + b2)                   # [B,S,H]
    enc   = h2 @ W3 + b3                         # [B,S,C]
    codes = sum_s enc * mask                     # [B,C]
    out   = (tanh(tanh(codes@W4+b4)@W5+b5)) @ W6 + b6   # [B,T]

Algebraic collapse: codes = (sum_s mask*h2) @ W3 + N_b * b3, so only the two
big MLP layers run on device; the tiny decode runs on host.

Layout strategy (v2): BOTH layers produce transposed activations [h, r]
(h on partitions, rows moving).  This makes the per-feature biases b1/b2
per-PARTITION quantities, so the ScalarE activation instruction applies
bias+tanh in one pass — no VectorE bias add at all.  The segment sum
(rows are packed contiguously per set) becomes a free-axis segmented
reduction over a2[h, r] handled by the idle VectorE — the 38 PE segment
matmuls of v1 are gone.  Everything streams in bf16 (rel err ~3e-3,
gate is 2e-2): halves input DMA and makes LDWEIGHTS (150ns) fully
hidden behind each 512-moving matmul (fp32 LDWEIGHTS was 330ns > 213ns
matmul and leaked into the critical path).

Per 512-row tile (R rows/core, packed valid rows, zero pad):
    L1: for m (h-chunk): ps1[m][h,r] = sum_k W1[k,m]^T a0[k]   (4 mm)
        a1[m] = tanh(ps1[m] + b1[m])        ScalarE, per-partition bias
    L2: for m: for n: ps2[n][h,r] += W2[m,n]^T a1[m]           (16 mm)
        a2[n] = tanh(ps2[n] + b2[n])        ScalarE
    seg: for each set-piece [lo,hi) in tile: VectorE reduce_sum over
        a2[n][:, lo:hi] -> outsb[:, n, t, j]; host scatters to sets.
PSUM: 4 banks L1 + 4 banks L2 = 8 (all).  m-outer ordering lets L2(t)
matmuls start while L1(t) activations finish (no PE bubble).
"""

import sys

if "/opt/trn_rl_repo" not in sys.path:
    sys.path.insert(0, "/opt/trn_rl_repo")

import ml_dtypes
import numpy as np

import concourse.bass as bass
import concourse.mybir as mybir
import concourse.tile as tile
from concourse import bacc
from concourse.bass_utils import run_bass_kernel_spmd

B, S, E = 64, 1024, 512
H = 512
NCORES = 8
P = 128
RT = 512  # rows per row-tile (matmul moving dim)
KC = E // P  # 4 contraction chunks

N_WARMUP = 8  # dep-free matmuls to open the HAM clock gate during DMA wait

f32 = mybir.dt.float32
bf16 = mybir.dt.bfloat16

_cache: dict = {}


def _tiles_of(R: int):
    assert R % 256 == 0 and R >= RT
    return [RT] * (R // RT) + ([256] if R % RT else [])


def _build(R: int, pieces_key):
    """pieces_key: tuple over tiles of tuples (lo, hi) per piece."""
    key = (R, pieces_key)
    if key in _cache:
        return _cache[key]

    tiles = _tiles_of(R)
    nt = len(tiles)
    offs = [sum(tiles[:i]) for i in range(nt)]
    pmax = max(1, max((len(p) for p in pieces_key), default=1))

    nc = bacc.Bacc("TRN2", target_bir_lowering=False, debug=False, num_devices=NCORES)

    wT_d = nc.dram_tensor("wT", [P, KC, R], bf16, kind="ExternalInput").ap()
    # tile-0 words again, but contiguous per partition: DMA lines are 2KB
    # (vs 1KB slicing wT), ~1.5x the startup bandwidth where it matters
    wh_d = nc.dram_tensor("wh", [P, KC, RT], bf16, kind="ExternalInput").ap()
    w1_d = nc.dram_tensor("w1", [P, KC, H], bf16, kind="ExternalInput").ap()
    w2_d = nc.dram_tensor("w2", [P, KC, H], bf16, kind="ExternalInput").ap()
    bias_d = nc.dram_tensor("bias", [P, 2, KC], f32, kind="ExternalInput").ap()
    out_d = nc.dram_tensor("hsum", [nt, P, KC, pmax], f32, kind="ExternalOutput").ap()

    with tile.TileContext(nc) as tc:
        with (
            tc.tile_pool(name="const", bufs=1) as cpool,
            tc.tile_pool(name="a0", bufs=3) as a0pool,
            tc.tile_pool(name="a1", bufs=2) as a1pool,
            tc.tile_pool(name="a2", bufs=3) as a2pool,
            tc.tile_pool(name="ps1", bufs=4, space="PSUM") as ps1pool,
            tc.tile_pool(name="ps2", bufs=4, space="PSUM") as ps2pool,
        ):
            # --- DMA issue order = critical path first ---------------------
            # Sync engine issues DMA descriptors serially (~0.65us each).
            # Biases (one tiny 4KB transfer) go first: the very first tanh
            # needs them.  Then the first L1 matmul's deps (w1[k0],
            # a0[t0][k0]) in k-pair order matching the t0 k-outer schedule.
            bsb = cpool.tile([P, 2, KC], f32)
            nc.sync.dma_start(bsb[:], bias_d)
            b1sb = bsb[:, 0]
            b2sb = bsb[:, 1]
            # startup-critical tensors move as chunk-PAIR transfers with
            # contiguous 2KB/partition lines, interleaved in the order the
            # t0 k-outer matmul schedule consumes them
            w1h = []
            a0h = []
            for g in range(2):
                w1c = cpool.tile([P, 2, H], bf16, name=f"w1h{g}")
                nc.sync.dma_start(w1c[:], w1_d[:, 2 * g:2 * g + 2, :])
                w1h.append(w1c)
                a0c = cpool.tile([P, 2, RT], bf16, name=f"a0h{g}")
                nc.sync.dma_start(a0c[:], wh_d[:, 2 * g:2 * g + 2, :])
                a0h.append(a0c)
            w2h = []
            for g in range(2):
                w2c = cpool.tile([P, 2, H], bf16, name=f"w2h{g}")
                nc.sync.dma_start(w2c[:], w2_d[:, 2 * g:2 * g + 2, :])
                w2h.append(w2c)

            def w1k(k):
                return w1h[k // 2][:, k % 2]

            def w2k(m):
                return w2h[m // 2][:, m % 2]
            # remaining row-tiles: one combined 4-chunk DMA each; the pool
            # (bufs=3) back-pressures the Sync queue harmlessly — everything
            # urgent is already issued above.
            a0t = {}
            for t in range(1, nt):
                a0c = a0pool.tile([P, KC, RT], bf16, tag="a0", name=f"a0t{t}")
                nc.sync.dma_start(
                    a0c[:, :, :tiles[t]], wT_d[:, :, offs[t]:offs[t] + tiles[t]]
                )
                a0t[t] = a0c

            # --- PE warmup: open the HAM clock gate during DMA wait --------
            # (measured: the gate opens ~4.3us after ~3.4us of continuous PE
            # activity, right as the first input DMAs land)
            warm_sb = cpool.tile([P, RT], bf16)
            nc.gpsimd.memset(warm_sb[:], 0.25)
            # dummy tanh: pulls the 1.3us ACT_TABLE_LOAD off the critical
            # path (otherwise it runs inline in the first real activation)
            warm_act = cpool.tile([P, 1], f32)
            nc.scalar.activation(
                warm_act[:], warm_sb[:, :1], mybir.ActivationFunctionType.Tanh
            )
            for w in range(N_WARMUP):
                wps = ps1pool.tile([P, RT], f32, tag="ps1", name="wps")
                nc.tensor.matmul(
                    wps[:], warm_sb[:, :P], warm_sb[:], start=True, stop=True
                )

            # --- main pipeline --------------------------------------------
            for t in range(nt):
                nr = tiles[t]
                pieces = pieces_key[t]
                # L1 -> a1[m] = tanh(W1[:,m]^T a0 + b1[m]), layout [h, r]
                a1 = [
                    a1pool.tile([P, RT], bf16, tag=f"a1m{m}", name=f"a1m{m}")
                    for m in range(KC)
                ]
                if t == 0:
                    # k-outer: each arriving (w1[k], a0[k]) DMA pair feeds 4
                    # matmuls immediately
                    pss = [
                        ps1pool.tile([P, RT], f32, tag="ps1", name=f"ps1_{m}")
                        for m in range(KC)
                    ]
                    for k in range(KC):
                        for m in range(KC):
                            nc.tensor.matmul(
                                pss[m][:, :nr],
                                w1k(k)[:, m * P:(m + 1) * P],
                                a0h[k // 2][:, k % 2, :nr],
                                start=(k == 0),
                                stop=(k == KC - 1),
                            )
                    for m in range(KC):
                        nc.scalar.activation(
                            a1[m][:, :nr], pss[m][:, :nr],
                            mybir.ActivationFunctionType.Tanh,
                            bias=b1sb[:, m:m + 1],
                        )
                else:
                    a0c = a0t[t]
                    for m in range(KC):
                        ps = ps1pool.tile([P, RT], f32, tag="ps1")
                        for k in range(KC):
                            nc.tensor.matmul(
                                ps[:, :nr],
                                w1k(k)[:, m * P:(m + 1) * P],
                                a0c[:, k, :nr],
                                start=(k == 0),
                                stop=(k == KC - 1),
                            )
                        nc.scalar.activation(
                            a1[m][:, :nr], ps[:, :nr],
                            mybir.ActivationFunctionType.Tanh,
                            bias=b1sb[:, m:m + 1],
                        )
                # L2: m-outer so the first matmuls only need a1[0] (no PE
                # bubble waiting on L1 activations).  The last tile runs
                # n-outer instead: each psum bank finishes early so its
                # activation overlaps remaining matmuls (shorter kernel tail;
                # a1 is fully ready there anyway).
                ps2 = [
                    ps2pool.tile([P, RT], f32, tag="ps2", name=f"ps2_{n}")
                    for n in range(KC)
                ]
                a2 = a2pool.tile([P, KC, RT], bf16, tag="a2", name="a2")
                if t == nt - 1:
                    for n in range(KC):
                        for m in range(KC):
                            nc.tensor.matmul(
                                ps2[n][:, :nr],
                                w2k(m)[:, n * P:(n + 1) * P],
                                a1[m][:, :nr],
                                start=(m == 0),
                                stop=(m == KC - 1),
                            )
                        nc.scalar.activation(
                            a2[:, n, :nr], ps2[n][:, :nr],
                            mybir.ActivationFunctionType.Tanh,
                            bias=b2sb[:, n:n + 1],
                        )
                else:
                    for m in range(KC):
                        for n in range(KC):
                            nc.tensor.matmul(
                                ps2[n][:, :nr],
                                w2k(m)[:, n * P:(n + 1) * P],
                                a1[m][:, :nr],
                                start=(m == 0),
                                stop=(m == KC - 1),
                            )
                    for n in range(KC):
                        nc.scalar.activation(
                            a2[:, n, :nr], ps2[n][:, :nr],
                            mybir.ActivationFunctionType.Tanh,
                            bias=b2sb[:, n:n + 1],
                        )
                # segment partial sums, all 4 h-chunks per instr (VectorE)
                outsb = a1pool.tile([P, KC, pmax], f32, tag="out", name="outsb")
                for j, (lo, hi) in enumerate(pieces):
                    nc.vector.reduce_sum(
                        outsb[:, :, j],
                        a2[:, :, lo:hi],
                        axis=mybir.AxisListType.X,
                    )
                nc.sync.dma_start(out_d[t], outsb[:])

    nc.compile()
    _cache[key] = nc
    return nc


def _pack(words: np.ndarray):
    """Pack valid rows contiguously, split across cores.

    Returns per-core (wT bf16 [P,KC,R], pieces [(lo,hi,gid) per tile]) plus
    R and the validity mask.
    """
    words = np.asarray(words, dtype=np.float32)
    mask = np.sign(np.abs(words.sum(axis=-1)))  # [B, S], matches reference
    valid = mask > 0

    rows = []
    segs = []
    for b in range(B):
        vb = words[b][valid[b]]
        rows.append(vb)
        segs.append(np.full(len(vb), b, dtype=np.int64))
    rows = np.concatenate(rows, axis=0)
    segs = np.concatenate(segs, axis=0)
    total = len(rows)

    # balanced split: per-core valid counts differ by <=1, so the tail
    # boundaries of all cores collapse to at most two span offsets
    base, extra = divmod(total, NCORES)
    counts = [base + (1 if c < extra else 0) for c in range(NCORES)]
    starts = np.concatenate([[0], np.cumsum(counts)])
    R = max(RT, -(-max(counts) // 256) * 256)  # pad to tile granularity
    tiles = _tiles_of(R)
    offs = [sum(tiles[:i]) for i in range(len(tiles))]

    per_core = []
    for c in range(NCORES):
        lo, hi = int(starts[c]), int(starts[c + 1])
        chunk = rows[lo:hi]
        seg_chunk = segs[lo:hi]
        n = hi - lo
        if n < R:
            chunk = np.concatenate(
                [chunk, np.zeros((R - n, E), dtype=np.float32)], axis=0
            )
        wT = np.ascontiguousarray(
            chunk.T.reshape(KC, P, R).transpose(1, 0, 2)
        ).astype(ml_dtypes.bfloat16)  # [P, KC, R]
        # set pieces per tile: maximal runs of equal set id among valid rows
        pieces = []
        for t, nr in enumerate(tiles):
            t_lo, t_hi = offs[t], min(offs[t] + nr, n)
            pt = []
            i = t_lo
            while i < t_hi:
                g = seg_chunk[i]
                j = i
                while j < t_hi and seg_chunk[j] == g:
                    j += 1
                pt.append((i - t_lo, j - t_lo, int(g)))
                i = j
            pieces.append(pt)
        per_core.append((wT, pieces))
    return per_core, R, mask


def _in_maps(per_core, inputs):
    W1 = np.asarray(inputs["W1"], dtype=np.float32)
    W2 = np.asarray(inputs["W2"], dtype=np.float32)
    b1 = np.asarray(inputs["b1"], dtype=np.float32)
    b2 = np.asarray(inputs["b2"], dtype=np.float32)
    w1r = np.ascontiguousarray(
        W1.reshape(KC, P, H).transpose(1, 0, 2)
    ).astype(ml_dtypes.bfloat16)  # [P, KC, H], [p,k,h] = W1[k*P+p, h]
    w2r = np.ascontiguousarray(
        W2.reshape(KC, P, H).transpose(1, 0, 2)
    ).astype(ml_dtypes.bfloat16)
    b1r = b1.reshape(KC, P).T  # [p, m] = b1[m*P+p]
    b2r = b2.reshape(KC, P).T
    biasr = np.ascontiguousarray(np.stack([b1r, b2r], axis=1))  # [P, 2, KC]
    return [
        {
            "wT": wT,
            "wh": np.ascontiguousarray(wT[:, :, :RT]),
            "w1": w1r,
            "w2": w2r,
            "bias": biasr,
        }
        for (wT, _pieces) in per_core
    ]


def _prepare(words):
    per_core, R, mask = _pack(words)
    # All cores share one SPMD program, but set boundaries differ per core.
    # Per tile position, collect the union of all cores' piece boundary
    # offsets and emit one reduce per consecutive-offset span: no span ever
    # straddles any core's set boundary, so each core's piece is an exact
    # union of spans (host sums the slots it needs and ignores the rest).
    pieces_key = []
    nt = len(_tiles_of(R))
    for t in range(nt):
        offsets = set()
        for (_wT, pieces) in per_core:
            for (lo, hi, _g) in pieces[t]:
                offsets.add(lo)
                offsets.add(hi)
        offs_sorted = sorted(offsets)
        spans = tuple(
            (a, b) for a, b in zip(offs_sorted, offs_sorted[1:])
        )
        pieces_key.append(spans)
    pieces_key = tuple(pieces_key)
    nc = _build(R, pieces_key)
    return per_core, pieces_key, R, mask, nc


def kernel(words, W1, b1, W2, b2, W3, b3, W4, b4, W5, b5, W6, b6):
    per_core, pieces_key, R, mask, nc = _prepare(words)
    in_maps = _in_maps(per_core, {"W1": W1, "W2": W2, "b1": b1, "b2": b2})

    res = run_bass_kernel_spmd(nc, in_maps, core_ids=list(range(NCORES)))

    hsum = np.zeros((B, H), dtype=np.float32)
    for c in range(NCORES):
        out_c = res.results[c]["hsum"]  # [nt, P, KC, pmax]
        hvec = out_c.transpose(2, 1, 0, 3).reshape(H, out_c.shape[0], -1)
        pieces = per_core[c][1]
        for t, pt in enumerate(pieces):
            for (lo, hi, g) in pt:
                for j, (slo, shi) in enumerate(pieces_key[t]):
                    if lo <= slo and shi <= hi:
                        hsum[g] += hvec[:, t, j]

    # host decode (tiny)
    lengths = mask.sum(axis=1).astype(np.float32)[:, None]
    codes = hsum @ np.asarray(W3, np.float32) + lengths * np.asarray(b3, np.float32)
    h = np.tanh(codes @ np.asarray(W4, np.float32) + np.asarray(b4, np.float32))
    h = np.tanh(h @ np.asarray(W5, np.float32) + np.asarray(b5, np.float32))
    out = h @ np.asarray(W6, np.float32) + np.asarray(b6, np.float32)
    return out.astype(np.float32)
